# revision 18
# baseline (speedup 1.0000x reference)
"""AdafuseNet multi-view heatmap fusion kernel for 8 TRN2 NeuronCores.

Pure data parallel: 32 batches sharded 4-per-core (8 bv-slices of
(17,128,128) heatmaps per core). Per core:
  stage 1: exp(hm/T) -> column/row sums via PE matmuls + transposes
           (soft-argmax), per-joint max via Pool reductions + DVE tree
  tiny math: 3x3 camera inverses, fundamental matrices, epipolar
           distances, view-weight sigmoid - strided DVE ops on [4,2,*]
  fusion:  fused = c0*hm0 + c1*hm1 (per-joint scalars)
  stage 2: soft-argmax on fused -> output coords
"""
import os
import sys

for _p in (
    "/root/.axon_site",
    "/root/.axon_site/_ro/trn_rl_repo",
    "/root/.axon_site/_ro/pypackages",
    "/opt/trn_rl_repo",
    "/opt/pypackages",
):
    if os.path.isdir(_p) and _p not in sys.path:
        sys.path.append(_p)

import numpy as np
import concourse.bass as bass
import concourse.tile as tile
from concourse import bacc
from concourse import mybir
from concourse.alu_op_type import AluOpType
from contextlib import ExitStack

B, V, J, H, W = 32, 2, 17, 128, 128
NC_ = 8
BPC = B // NC_        # 4 batches per core
S = BPC * V           # 8 bv-slices per core
TINV = 20.0           # 1 / softmax_temp
EPS = 1e-12
F32 = mybir.dt.float32
FD = J * W            # 2176 free elems per slice
X = mybir.AxisListType.X


def _ap(base, off, dims):
    """Custom free-dim AP on a tile: keep partition entry, replace free dims.
    dims = [[step, count], ...] in elements relative to base's offset."""
    b = base[:] if not isinstance(base, bass.AP) else base
    return bass.AP(tensor=b.tensor, offset=b.offset + off, ap=[list(b.ap[0])] + dims)


def _bcast_part(base, nparts=128):
    """Partition-broadcast view of a [1, ...] AP (step-0 partition dim)."""
    b = base[:] if not isinstance(base, bass.AP) else base
    return bass.AP(tensor=b.tensor, offset=b.offset,
                   ap=[[0, nparts]] + [list(d) for d in b.ap[1:]])


def build_nc():
    nc = bacc.Bacc()
    hms = nc.declare_dram_parameter("hms", [S, J, H, W], F32, isOutput=False)
    cst = nc.declare_dram_parameter("cst", [128, 340], F32, isOutput=False)
    out_hm = nc.declare_dram_parameter("out_hm", [S, J, H, W], F32, isOutput=True)
    out_img = nc.declare_dram_parameter("out_img", [BPC, V, 2, J], F32, isOutput=True)

    MU, ADD, SUB, MX, GT = (AluOpType.mult, AluOpType.add, AluOpType.subtract,
                            AluOpType.max, AluOpType.is_gt)
    ACT = mybir.ActivationFunctionType

    with tile.TileContext(nc) as tc, ExitStack() as ctx:
        consts = ctx.enter_context(tc.tile_pool(name="consts", bufs=1))
        big = ctx.enter_context(tc.tile_pool(name="big", bufs=1))
        epool = ctx.enter_context(tc.tile_pool(name="epool", bufs=2))
        fpool = ctx.enter_context(tc.tile_pool(name="fpool", bufs=2))
        e2pool = ctx.enter_context(tc.tile_pool(name="e2pool", bufs=2))
        sm = ctx.enter_context(tc.tile_pool(name="sm", bufs=1))
        ps_m1 = ctx.enter_context(tc.tile_pool(name="ps_m1", bufs=1, space="PSUM"))
        ps_t = ctx.enter_context(tc.tile_pool(name="ps_t", bufs=2, space="PSUM"))
        ps_p2 = ctx.enter_context(tc.tile_pool(name="ps_p2", bufs=1, space="PSUM"))

        cst_sb = consts.tile([128, 340], F32)
        nc.sync.dma_start(out=cst_sb, in_=cst[:])
        lhs1 = cst_sb[:, 0:2]          # [ones | arange]
        iden = cst_sb[:, 2:130]        # eye(128)
        lhs16 = lambda s: cst_sb[:, 130 + 16 * s:130 + 16 * (s + 1)]
        lhs8 = lambda b: cst_sb[:, 258 + 8 * b:258 + 8 * (b + 1)]
        zb = consts.tile([128, 1], F32)
        nc.vector.memset(zb, 0.0)

        # camera data packed in cst cols 290:340 on partitions 0-3
        K_cat = cst_sb[0:BPC, 290:308].rearrange("b (v e) -> b v e", v=V)
        T_cat = cst_sb[0:BPC, 308:340].rearrange("b (v e) -> b v e", v=V)

        # ---------------- stage 1: load, exp, maxcol, M1 ----------------
        hm_all = big.tile([128, S, J, W], F32)
        maxcol = big.tile([128, S, J, 1], F32)
        p_m1 = ps_m1.tile([2 * S, FD], F32, tag="m1")
        for s in range(S):
            nc.sync.dma_start(out=hm_all[:, s], in_=hms[s].rearrange("j h w -> h j w"))
            e_s = epool.tile([128, J, W], F32)
            nc.scalar.activation(out=e_s, in_=hm_all[:, s], func=ACT.Exp,
                                 bias=zb, scale=TINV)
            nc.vector.tensor_reduce(out=maxcol[:, s], in_=hm_all[:, s], axis=X, op=MX)
            ef = e_s.rearrange("p j w -> p (j w)")
            for c0 in range(0, FD, 512):
                c1 = min(c0 + 512, FD)
                nc.tensor.matmul(p_m1[:, c0:c1], lhs16(s), ef[:, c0:c1],
                                 start=(s == 0), stop=(s == S - 1))

        # maxv over partitions: PE-transpose maxcol halves, reduce over free axis,
        # then DMA the two [68,1] columns back into a [1,136] row
        mc = maxcol.rearrange("p s j r -> p (s j r)")
        mt1 = ps_t.tile([68, 128], F32, tag="pt")
        mt2 = ps_t.tile([68, 128], F32, tag="pt")
        nc.tensor.transpose(mt1, mc[:, 0:68], iden)
        nc.tensor.transpose(mt2, mc[:, 68:136], iden)
        mts = sm.tile([68, 2, 128], F32)
        nc.scalar.copy(out=mts[:, 0], in_=mt1)
        nc.scalar.copy(out=mts[:, 1], in_=mt2)
        mred = sm.tile([68, 2, 1], F32)
        nc.vector.tensor_reduce(out=mred, in_=mts, axis=X, op=MX)
        tree1 = sm.tile([1, S * J], F32)
        nc.sync.dma_start(out=tree1[:, 0:68], in_=mred[:, 0])
        nc.sync.dma_start(out=tree1[:, 68:136], in_=mred[:, 1])

        # m1 evacuation + per-joint transposes + M3
        m1_all = big.tile([2 * S, J, W], F32)
        nc.scalar.copy(out=m1_all.rearrange("p j w -> p (j w)"), in_=p_m1)
        p_t = ps_t.tile([128, J, 2 * S], F32, tag="pt")
        for j in range(J):
            nc.tensor.transpose(p_t[:, j], m1_all[:, j, :], iden[0:2 * S, 0:2 * S])
        csT = big.tile([128, J, 2 * S], F32)
        nc.scalar.copy(out=csT, in_=p_t)
        p_p2 = ps_p2.tile([2, J, S, 2], F32, tag="p2")
        nc.tensor.matmul(p_p2.rearrange("p j s r -> p (j s r)"),
                         lhs1, csT.rearrange("p j s -> p (j s)"),
                         start=True, stop=True)
        sums2 = sm.tile([2, S, 2, J], F32)
        # out iterates (j, s, r) to match p_p2's stream; writes s-major layout
        nc.scalar.copy(out=_ap(sums2, 0, [[1, J], [2 * J, S], [J, 2]]), in_=p_p2)

        # rearrange soft-argmax stats to [4, 2, 17]
        S_t = sm.tile([BPC, V, J], F32)
        ynum = sm.tile([BPC, V, J], F32)
        xnum = sm.tile([BPC, V, J], F32)
        conf = sm.tile([BPC, V, J], F32)
        # sums2 row layout: s*34 + r*17 + j  (contiguous j innermost)
        nc.sync.dma_start(out=S_t, in_=_ap(sums2[0:1], 0, [[2 * J, S], [1, J]]))
        nc.sync.dma_start(out=ynum, in_=_ap(sums2[0:1], J, [[2 * J, S], [1, J]]))
        nc.sync.dma_start(out=xnum, in_=_ap(sums2[1:2], 0, [[2 * J, S], [1, J]]))
        nc.sync.dma_start(out=conf, in_=_ap(tree1[0:1], 0, [[J, S], [1, J]]))

        rS = sm.tile([BPC, V, J], F32)
        nc.vector.reciprocal(rS, S_t)
        img = sm.tile([BPC, V, 3, J], F32)
        nc.vector.scalar_tensor_tensor(img[:, :, 0], xnum, 4.0, rS, op0=MU, op1=MU)
        nc.vector.scalar_tensor_tensor(img[:, :, 1], ynum, 4.0, rS, op0=MU, op1=MU)
        nc.vector.memset(img[:, :, 2], 1.0)

        # mv = where(conf > 0.01, conf, 1e6); inv_mv = 1/mv
        mask = sm.tile([BPC, V, J], F32)
        nc.vector.tensor_scalar(mask, conf, 0.01, None, op0=GT)
        mv = sm.tile([BPC, V, J], F32)
        nc.vector.tensor_tensor(mv, conf, mask, op=MU)
        mnot = sm.tile([BPC, V, J], F32)
        nc.vector.tensor_scalar(mnot, mask, -1e6, 1e6, op0=MU, op1=ADD)
        nc.vector.tensor_tensor(mv, mv, mnot, op=ADD)
        inv_mv = sm.tile([BPC, V, J], F32)
        nc.vector.reciprocal(inv_mv, mv)

        # ---------------- camera math ----------------
        # K4[a,b] = K[a%3, b%3]  -> [4, 2, 36] (row a: stride 6)
        K4 = sm.tile([BPC, V, 36], F32)
        src_K = _ap(K_cat, 0, [[9, V], [3, 3], [1, 3]])
        for qa, qb in ((0, 0), (0, 3), (3, 0), (3, 3)):
            nc.vector.tensor_copy(_ap(K4, qa * 6 + qb, [[36, V], [6, 3], [1, 3]]),
                                  src_K)
        # cofactor-transpose: C_T[m,n] = K4[n+1,m+1]K4[n+2,m+2] - K4[n+1,m+2]K4[n+2,m+1]
        u1 = sm.tile([BPC, V, 9], F32)
        u2 = sm.tile([BPC, V, 9], F32)
        cof = sm.tile([BPC, V, 9], F32)
        st = [[36, V], [1, 3], [6, 3]]  # over (v, m, n): m step 1, n step 6
        nc.vector.tensor_tensor(u1, _ap(K4, 7, st), _ap(K4, 14, st), op=MU)
        nc.vector.tensor_tensor(u2, _ap(K4, 8, st), _ap(K4, 13, st), op=MU)
        nc.vector.tensor_tensor(cof, u1, u2, op=SUB)
        det3 = sm.tile([BPC, V, 3], F32)
        nc.vector.tensor_tensor(det3, _ap(K_cat, 0, [[9, V], [1, 3]]),
                                _ap(cof, 0, [[9, V], [3, 3]]), op=MU)
        det = sm.tile([BPC, V, 1], F32)
        nc.vector.tensor_reduce(out=det, in_=det3, axis=X, op=ADD)
        rdet = sm.tile([BPC, V, 1], F32)
        nc.vector.reciprocal(rdet, det)
        invK = sm.tile([BPC, V, 9], F32)
        nc.vector.scalar_tensor_tensor(invK, cof, 1.0,
                                       _ap(rdet, 0, [[1, V], [0, 9]]),
                                       op0=MU, op1=MU)
        invK_sw = sm.tile([BPC, V, 9], F32)
        nc.vector.tensor_copy(invK_sw[:, 0], invK[:, 1])
        nc.vector.tensor_copy(invK_sw[:, 1], invK[:, 0])

        # r01[m,n] = sum_k R0[m,k] R1[n,k];  R[m,k] = T_cat[v, 4m+k]
        r01 = sm.tile([BPC, 9], F32)
        tmp9 = sm.tile([BPC, 9], F32)
        for k in range(3):
            dst = r01 if k == 0 else tmp9
            nc.vector.tensor_tensor(dst, _ap(T_cat, k, [[4, 3], [0, 3]]),
                                    _ap(T_cat, 16 + k, [[0, 3], [4, 3]]), op=MU)
            if k:
                nc.vector.tensor_tensor(r01, r01, tmp9, op=ADD)

        # t_pair[p] = t_i - r_p @ t_j ; r_p1 = r01^T
        t_t = sm.tile([BPC, V, 3], F32)
        tmp33 = sm.tile([BPC, 3, 3], F32)
        tmp3 = sm.tile([BPC, 3, 1], F32)
        nc.vector.tensor_tensor(tmp33, _ap(r01, 0, [[3, 3], [1, 3]]),
                                _ap(T_cat, 16 + 3, [[0, 3], [4, 3]]), op=MU)
        nc.vector.tensor_reduce(out=tmp3, in_=tmp33, axis=X, op=ADD)
        nc.vector.tensor_tensor(t_t[:, 0], _ap(T_cat, 3, [[4, 3]]),
                                tmp3[:, :, 0], op=SUB)
        nc.vector.tensor_tensor(tmp33, _ap(r01, 0, [[1, 3], [3, 3]]),
                                _ap(T_cat, 3, [[0, 3], [4, 3]]), op=MU)
        nc.vector.tensor_reduce(out=tmp3, in_=tmp33, axis=X, op=ADD)
        nc.vector.tensor_tensor(t_t[:, 1], _ap(T_cat, 16 + 3, [[4, 3]]),
                                tmp3[:, :, 0], op=SUB)

        # M[m,n] = t[m+1] r[m+2, n] - t[m+2] r[m+1, n]   (tiled copies)
        t_t2 = sm.tile([BPC, V, 6], F32)
        nc.vector.tensor_copy(_ap(t_t2, 0, [[6, V], [1, 3]]), t_t)
        nc.vector.tensor_copy(_ap(t_t2, 3, [[6, V], [1, 3]]), t_t)
        r_t2 = sm.tile([BPC, V, 18], F32)
        nc.vector.tensor_copy(_ap(r_t2, 0, [[1, 9]]), r01)
        nc.vector.tensor_copy(_ap(r_t2, 9, [[1, 9]]), r01)
        rT = _ap(r01, 0, [[1, 3], [3, 3]])
        nc.vector.tensor_copy(_ap(r_t2, 18, [[3, 3], [1, 3]]), rT)
        nc.vector.tensor_copy(_ap(r_t2, 27, [[3, 3], [1, 3]]), rT)
        Mu1 = sm.tile([BPC, V, 9], F32)
        Mu2 = sm.tile([BPC, V, 9], F32)
        Mcat = sm.tile([BPC, V, 9], F32)
        nc.vector.tensor_tensor(Mu1, _ap(t_t2, 1, [[6, V], [1, 3], [0, 3]]),
                                _ap(r_t2, 6, [[18, V], [3, 3], [1, 3]]), op=MU)
        nc.vector.tensor_tensor(Mu2, _ap(t_t2, 2, [[6, V], [1, 3], [0, 3]]),
                                _ap(r_t2, 3, [[18, V], [3, 3], [1, 3]]), op=MU)
        nc.vector.tensor_tensor(Mcat, Mu1, Mu2, op=SUB)

        # G = invK_i^T @ M ; F = G @ invK_j
        G = sm.tile([BPC, V, 9], F32)
        tmpG = sm.tile([BPC, V, 9], F32)
        for p_ in range(3):
            dst = G if p_ == 0 else tmpG
            nc.vector.tensor_tensor(dst, _ap(invK, 3 * p_, [[9, V], [1, 3], [0, 3]]),
                                    _ap(Mcat, 3 * p_, [[9, V], [0, 3], [1, 3]]), op=MU)
            if p_:
                nc.vector.tensor_tensor(G, G, tmpG, op=ADD)
        Fm = sm.tile([BPC, V, 9], F32)
        tmpF = sm.tile([BPC, V, 9], F32)
        for q in range(3):
            dst = Fm if q == 0 else tmpF
            nc.vector.tensor_tensor(dst, _ap(G, q, [[9, V], [3, 3], [0, 3]]),
                                    _ap(invK_sw, 3 * q, [[9, V], [0, 3], [1, 3]]),
                                    op=MU)
            if q:
                nc.vector.tensor_tensor(Fm, Fm, tmpF, op=ADD)

        # epipolar distances
        img_sw = sm.tile([BPC, V, 3, J], F32)
        nc.vector.tensor_copy(img_sw[:, 0], img[:, 1])
        nc.vector.tensor_copy(img_sw[:, 1], img[:, 0])
        l_t = sm.tile([BPC, V, 3, J], F32)
        l_tmp = sm.tile([BPC, V, 3, J], F32)
        for n in range(3):
            dst = l_t if n == 0 else l_tmp
            nc.vector.tensor_tensor(dst, _ap(Fm, n, [[9, V], [3, 3], [0, J]]),
                                    _ap(img_sw, J * n, [[3 * J, V], [0, 3], [1, J]]),
                                    op=MU)
            if n:
                nc.vector.tensor_tensor(l_t, l_t, l_tmp, op=ADD)
        stile = sm.tile([BPC, V, 3, J], F32)
        nc.vector.tensor_tensor(stile, img, l_t, op=MU)
        snum = sm.tile([BPC, V, J], F32)
        nc.vector.tensor_tensor(snum, stile[:, :, 0], stile[:, :, 1], op=ADD)
        nc.vector.tensor_tensor(snum, snum, stile[:, :, 2], op=ADD)
        lp = sm.tile([BPC, V, 2, J], F32)
        lp_tmp = sm.tile([BPC, V, 2, J], F32)
        for n in range(3):
            dst = lp if n == 0 else lp_tmp
            nc.vector.tensor_tensor(dst, _ap(Fm, 3 * n, [[9, V], [1, 2], [0, J]]),
                                    _ap(img, J * n, [[3 * J, V], [0, 2], [1, J]]),
                                    op=MU)
            if n:
                nc.vector.tensor_tensor(lp, lp, lp_tmp, op=ADD)
        q1 = sm.tile([BPC, V, 2, J], F32)
        nc.vector.tensor_tensor(q1, l_t[:, :, 0:2], l_t[:, :, 0:2], op=MU)
        q2 = sm.tile([BPC, V, 2, J], F32)
        nc.vector.tensor_tensor(q2, lp, lp, op=MU)
        nc.vector.tensor_tensor(q1, q1, q2, op=ADD)
        div = sm.tile([BPC, V, J], F32)
        nc.vector.tensor_tensor(div, q1[:, :, 0], q1[:, :, 1], op=ADD)
        nsq = sm.tile([BPC, V, J], F32)
        nc.vector.tensor_tensor(nsq, snum, snum, op=MU)
        nc.vector.tensor_scalar(div, div, EPS, None, op0=ADD)
        rdiv = sm.tile([BPC, V, J], F32)
        nc.vector.reciprocal(rdiv, div)
        nc.vector.tensor_tensor(nsq, nsq, rdiv, op=MU)
        dist = sm.tile([BPC, V, J], F32)
        nc.scalar.activation(out=dist, in_=nsq, func=ACT.Sqrt, bias=zb[0:BPC], scale=1.0)

        # score, view-softmax (sigmoid of score diff), fusion coefs
        score = sm.tile([BPC, V, J], F32)
        nc.vector.tensor_tensor(score, conf, dist, op=SUB)
        score_sw = sm.tile([BPC, V, J], F32)
        nc.vector.tensor_copy(score_sw[:, 0], score[:, 1])
        nc.vector.tensor_copy(score_sw[:, 1], score[:, 0])
        sd = sm.tile([BPC, V, J], F32)
        nc.vector.tensor_tensor(sd, score, score_sw, op=SUB)
        vw = sm.tile([BPC, V, J], F32)
        nc.scalar.activation(out=vw, in_=sd, func=ACT.Sigmoid, bias=zb[0:BPC], scale=1.0)
        c_t = sm.tile([BPC, V, J], F32)
        nc.vector.tensor_tensor(c_t, vw, inv_mv, op=MU)

        # broadcast c to all partitions: gather to one row, then PE outer product
        c_row = sm.tile([1, S * J], F32)
        nc.sync.dma_start(out=c_row, in_=c_t)
        ones_row = consts.tile([1, 128], F32)
        nc.vector.memset(ones_row, 1.0)
        p_bc = ps_t.tile([128, S * J], F32, tag="pt")
        nc.tensor.matmul(p_bc, ones_row, c_row, start=True, stop=True)
        c_bc = sm.tile([128, S * J], F32)
        nc.scalar.copy(out=c_bc, in_=p_bc)

        # ---------------- fusion + stage 2 ----------------
        p_m2 = ps_m1.tile([2 * S, FD], F32, tag="m1")
        for b in range(BPC):
            fused = fpool.tile([128, J, W], F32)
            cb0 = _ap(c_bc, (b * V + 0) * J, [[1, J], [0, W]])
            cb1 = _ap(c_bc, (b * V + 1) * J, [[1, J], [0, W]])
            nc.vector.tensor_tensor(fused, hm_all[:, 2 * b], cb0, op=MU)
            tmpf = fpool.tile([128, J, W], F32)
            nc.gpsimd.tensor_tensor(tmpf, hm_all[:, 2 * b + 1], cb1, op=MU)
            nc.vector.tensor_tensor(fused, fused, tmpf, op=ADD)
            nc.sync.dma_start(out=out_hm[2 * b].rearrange("j h w -> h j w"), in_=fused)
            nc.sync.dma_start(out=out_hm[2 * b + 1].rearrange("j h w -> h j w"),
                              in_=fused)
            e2 = e2pool.tile([128, J, W], F32)
            nc.scalar.activation(out=e2, in_=fused, func=ACT.Exp, bias=zb, scale=TINV)
            e2f = e2.rearrange("p j w -> p (j w)")
            for c0 in range(0, FD, 512):
                c1 = min(c0 + 512, FD)
                nc.tensor.matmul(p_m2[0:2 * BPC, c0:c1], lhs8(b), e2f[:, c0:c1],
                                 start=(b == 0), stop=(b == BPC - 1))

        m2_all = big.tile([2 * BPC, J, W], F32)
        nc.scalar.copy(out=m2_all.rearrange("p j w -> p (j w)"), in_=p_m2[0:2 * BPC])
        p_t2 = ps_t.tile([128, J, 2 * BPC], F32, tag="pt")
        for j in range(J):
            nc.tensor.transpose(p_t2[:, j], m2_all[:, j, :],
                                iden[0:2 * BPC, 0:2 * BPC])
        csT2 = big.tile([128, J, 2 * BPC], F32)
        nc.scalar.copy(out=csT2, in_=p_t2)
        p_p2b = ps_p2.tile([2, J, BPC, 2], F32, tag="p2")
        nc.tensor.matmul(p_p2b.rearrange("p j b r -> p (j b r)"), lhs1,
                         csT2.rearrange("p j c -> p (j c)"), start=True, stop=True)
        sums2b = sm.tile([2, BPC, 2, J], F32)
        nc.scalar.copy(out=_ap(sums2b, 0, [[1, J], [2 * J, BPC], [J, 2]]), in_=p_p2b)

        S2 = sm.tile([BPC, J], F32)
        y2n = sm.tile([BPC, J], F32)
        x2n = sm.tile([BPC, J], F32)
        nc.sync.dma_start(out=S2, in_=_ap(sums2b[0:1], 0, [[2 * J, BPC], [1, J]]))
        nc.sync.dma_start(out=y2n, in_=_ap(sums2b[0:1], J, [[2 * J, BPC], [1, J]]))
        nc.sync.dma_start(out=x2n, in_=_ap(sums2b[1:2], 0, [[2 * J, BPC], [1, J]]))
        rS2 = sm.tile([BPC, J], F32)
        nc.vector.reciprocal(rS2, S2)
        imgc = sm.tile([BPC, 2, J], F32)
        nc.vector.scalar_tensor_tensor(imgc[:, 0], x2n, 4.0, rS2, op0=MU, op1=MU)
        nc.vector.scalar_tensor_tensor(imgc[:, 1], y2n, 4.0, rS2, op0=MU, op1=MU)
        nc.sync.dma_start(out=out_img[:, 0], in_=imgc)
        nc.sync.dma_start(out=out_img[:, 1], in_=imgc)

    nc.finalize()
    return nc


def _make_cst(apk=None, apt=None, latk=None, latt=None):
    cst = np.zeros((128, 340), dtype=np.float32)
    if apk is not None:
        cst[0:BPC, 290:299] = apk.reshape(BPC, 9)
        cst[0:BPC, 299:308] = latk.reshape(BPC, 9)
        cst[0:BPC, 308:324] = apt.reshape(BPC, 16)
        cst[0:BPC, 324:340] = latt.reshape(BPC, 16)
    cst[:, 0] = 1.0
    cst[:, 1] = np.arange(128, dtype=np.float32)
    cst[:, 2:130] = np.eye(128, dtype=np.float32)
    for s in range(S):
        cst[:, 130 + 16 * s + 2 * s] = 1.0
        cst[:, 130 + 16 * s + 2 * s + 1] = np.arange(128, dtype=np.float32)
    for b in range(BPC):
        cst[:, 258 + 8 * b + 2 * b] = 1.0
        cst[:, 258 + 8 * b + 2 * b + 1] = np.arange(128, dtype=np.float32)
    return cst


_NC_CACHE = []
LAST_RESULTS = None


def kernel(origin_hms, AP_K, AP_T, LAT_K, LAT_T):
    global LAST_RESULTS
    from concourse.bass_utils import run_bass_kernel_spmd
    if not _NC_CACHE:
        _NC_CACHE.append(build_nc())
    nc = _NC_CACHE[0]
    f32c = lambda a: np.ascontiguousarray(np.asarray(a), dtype=np.float32)
    in_maps = []
    for c in range(NC_):
        bs = slice(BPC * c, BPC * (c + 1))
        in_maps.append({
            "hms": f32c(origin_hms[S * c:S * (c + 1)]),
            "cst": _make_cst(f32c(AP_K[bs]), f32c(AP_T[bs]),
                             f32c(LAT_K[bs]), f32c(LAT_T[bs])),
        })
    trace = os.environ.get("BASS_KERNEL_TRACE", "0") == "1"
    res = run_bass_kernel_spmd(nc, in_maps, core_ids=list(range(NC_)), trace=trace)
    LAST_RESULTS = res
    img2 = np.empty((B, V, 2, J), dtype=np.float32)
    fused = np.empty((B * V, J, H, W), dtype=np.float32)
    for c in range(NC_):
        img2[BPC * c:BPC * (c + 1)] = res.results[c]["out_img"]
        fused[S * c:S * (c + 1)] = res.results[c]["out_hm"]
    return img2, fused
